# revision 23
# baseline (speedup 1.0000x reference)
"""Trainium2 Bass kernel for nn_CrossAttentionT2S (fused pos-embed cross-attention).

Sharding: data-parallel over the true batch axis b=8, one batch element per
NeuronCore. All tensors bf16 on device; feature-major ("transposed",
[feature, token]) layouts so matmuls contract over the partition dim.

Per core (NT=1568 q tokens, 1568 kv tokens, 12 heads, dh=64):
  tT = t_xT + pos_tT ; sT = s_xT + pos_sT          (DVE bf16 adds)
  kT = k_w @ t (feature-major, 6 chunks of 2 heads) (PE, evac on DVE + bias)
  qT = (q_w*SCALE) @ s + q_b*SCALE                  (PE, evac DVE)
  V' = token-major [128tok, parity, 6, 128]: even heads [v|ones64],
       odd heads [ones64|v]                          (PE, evac DVE, ones memset)
  per (qb in 512,512,512 + rump32, c6 head-pair, ki in 13):
    S[k128, q512]x2 heads — two row-tiled matmuls (0,0)/(64,0), concurrent
    P = exp(S) — ONE ScalarE activation per (c6,qb,ki), [128, 1024] free
    O~/den: AV matmul lhsT=V'[h] M=128: 64 cols of v + 64 ones columns ->
       psum [128,512]: O~ on one 64-partition half, den replicated on other
    evac: DVE reciprocal_approx_fast(den half) -> rcp, DVE mul -> OT bf16
  out = proj_w @ O + proj_b (PE, interleaved with next qb; DVE evac, DMA out)

ScalarE runs ONLY exp (the structural bottleneck ~275us); everything else is
kept off it. Projections/out-proj are emitted interleaved with attention so
the PE fills its exp-wait gaps and ACT never idles after warmup.
"""
import sys
import types
from contextlib import ExitStack

import numpy as np
import ml_dtypes

import concourse.bass as bass
import concourse.mybir as mybir
import concourse.tile as tile
from concourse import bacc
from concourse.bass_utils import run_bass_kernel_spmd

# ---------------------------------------------------------------- constants
DIM = 768
H = 12
DH = 64
T = 8
TS = 8
APATCH = 196
VP = 196
B = 8
NT = APATCH * TS          # 1568 tokens per core, both q and kv side
SCALE = DH ** -0.5
NCH = DIM // 128          # 6 feature chunks (2 heads each)
KT = 13                   # k tiles: 12 full 128 + rump 32
KR = NT - 12 * 128        # 32
QB = 512                  # q block
NQB = 3                   # full q blocks; rump = 32
QR = NT - NQB * QB        # 32
F32 = mybir.dt.float32
BF16 = mybir.dt.bfloat16
ADD = mybir.AluOpType.add
MULT = mybir.AluOpType.mult

_NC_CACHE = {}

import os
V_RECIP = os.environ.get("KV_RECIP", "fast")     # fast | exact
V_MEMSET = os.environ.get("KV_MEMSET", "pool")   # pool | dve
V_RUMP = os.environ.get("KV_RUMP", "1") == "1"
V_QALL = os.environ.get("KV_QALL", "0") == "1"
V_SERIAL = os.environ.get("KV_SERIAL", "0") == "1"



def kw_of(ki):
    return 128 if ki < 12 else KR


def build_nc():
    nc = bacc.Bacc(None)

    s_xT = nc.dram_tensor("s_xT", [DIM, NT], BF16, kind="ExternalInput")
    t_xT = nc.dram_tensor("t_xT", [DIM, NT], BF16, kind="ExternalInput")
    sp_s = nc.dram_tensor("sp_s", [DIM, APATCH], BF16, kind="ExternalInput")
    tp_s = nc.dram_tensor("tp_s", [DIM, TS], BF16, kind="ExternalInput")
    sp_t = nc.dram_tensor("sp_t", [DIM, VP], BF16, kind="ExternalInput")
    tp_t = nc.dram_tensor("tp_t", [DIM, T], BF16, kind="ExternalInput")
    q_wT = nc.dram_tensor("q_wT", [DIM, DIM], BF16, kind="ExternalInput")
    k_wT = nc.dram_tensor("k_wT", [DIM, DIM], BF16, kind="ExternalInput")
    v_wT = nc.dram_tensor("v_wT", [DIM, DIM], BF16, kind="ExternalInput")
    proj_wT = nc.dram_tensor("proj_wT", [DIM, DIM], BF16, kind="ExternalInput")
    q_b2 = nc.dram_tensor("q_b2", [128, NCH], F32, kind="ExternalInput")
    k_b2 = nc.dram_tensor("k_b2", [128, NCH], F32, kind="ExternalInput")
    p_b2 = nc.dram_tensor("p_b2", [128, NCH], F32, kind="ExternalInput")
    v_br = nc.dram_tensor("v_br", [128, DIM], F32, kind="ExternalInput")
    outT = nc.dram_tensor("outT", [DIM, NT], BF16, kind="ExternalOutput")
    DBG = os.environ.get("KV_DBG", "0") == "1"
    if DBG:
        dbg_qT = nc.dram_tensor("dbg_qT", [NCH, 128, NT], BF16, kind="ExternalOutput")
        dbg_kT = nc.dram_tensor("dbg_kT", [128, NT], BF16, kind="ExternalOutput")
        dbg_vP = nc.dram_tensor("dbg_vP", [128, 2, NCH, 128], BF16, kind="ExternalOutput")
        dbg_OT = nc.dram_tensor("dbg_OT", [NCH, 128, NT], BF16, kind="ExternalOutput")
        dbg_P = nc.dram_tensor("dbg_P", [128, 2, 512], BF16, kind="ExternalOutput")
        dbg_sT = nc.dram_tensor("dbg_sT", [128, NT], BF16, kind="ExternalOutput")

    with tile.TileContext(nc) as tc, ExitStack() as top:
        # ---------------- constant / persistent tiles
        cpool = top.enter_context(tc.tile_pool(name="consts", bufs=1))
        qb_t = cpool.tile([128, NCH], F32, tag="qb")
        kb_t = cpool.tile([128, NCH], F32, tag="kb")
        pb_t = cpool.tile([128, NCH], F32, tag="pb")
        vb_t = cpool.tile([128, DIM], F32, tag="vb")
        nc.sync.dma_start(qb_t[:], q_b2[:])
        nc.sync.dma_start(kb_t[:], k_b2[:])
        nc.sync.dma_start(pb_t[:], p_b2[:])
        nc.sync.dma_start(vb_t[:], v_br[:])

        w_pool = top.enter_context(tc.tile_pool(name="w", bufs=NCH))
        qw = [w_pool.tile([128, DIM], BF16, tag="qw", name=f"qw{c}") for c in range(NCH)]
        kw = [w_pool.tile([128, DIM], BF16, tag="kw", name=f"kw{c}") for c in range(NCH)]
        vw = [w_pool.tile([128, DIM], BF16, tag="vw", name=f"vw{c}") for c in range(NCH)]
        pw = [w_pool.tile([128, DIM], BF16, tag="pw", name=f"pw{c}") for c in range(NCH)]


        # psum pools + proj emitter (needed inside the prologue)
        qk_psum = top.enter_context(tc.tile_pool(name="qkps", bufs=2, space="PSUM"))
        av_psum = top.enter_context(tc.tile_pool(name="avps", bufs=2, space="PSUM"))
        gm_psum = top.enter_context(tc.tile_pool(name="gmps", bufs=2, space="PSUM"))
        QSL = [(i * QB, QB) for i in range(NQB)] + [(NQB * QB, QR)]

        def emit_proj(ws, xsrc, dst, bias_t, c_out, q0, qn):
            """dst[c_out][:, q0:q0+qn] = ws.T @ x (+bias), bf16 evac on DVE."""
            ps = gm_psum.tile([128, 512], F32, tag="gm")
            for c in range(NCH):
                nc.tensor.matmul(
                    ps[:, 0:qn],
                    ws[c][:, c_out * 128:(c_out + 1) * 128],
                    xsrc[c][:, q0:q0 + qn],
                    start=(c == 0), stop=(c == NCH - 1),
                )
            nc.vector.tensor_scalar_add(
                dst[c_out][:, q0:q0 + qn], ps[:, 0:qn], bias_t[:, c_out:c_out + 1]
            )

        # q/k feature-major bf16; V' token-major bf16 with ones blocks
        qkT_pool = top.enter_context(tc.tile_pool(name="qkT", bufs=NCH))
        qT = [qkT_pool.tile([128, NT], BF16, tag="qT", name=f"qT{c}") for c in range(NCH)]
        kT = [qkT_pool.tile([128, NT], BF16, tag="kT", name=f"kT{c}") for c in range(NCH)]
        vP_pool = top.enter_context(tc.tile_pool(name="vP", bufs=KT))
        # layout: [tok, parity, pair, 128]; head h = 2*pair+parity
        vP = [vP_pool.tile([128, 2, NCH, 128], BF16, tag="vP", name=f"vP{k}")
              for k in range(KT)]
        OT_pool = top.enter_context(tc.tile_pool(name="OT", bufs=NCH))
        OT = [OT_pool.tile([128, NT], BF16, tag="OT", name=f"OT{c}") for c in range(NCH)]

        # x + pos, bf16 feature-major
        xs_pool = top.enter_context(tc.tile_pool(name="xs", bufs=NCH))
        sT = [xs_pool.tile([128, NT], BF16, tag="sT", name=f"sT{c}") for c in range(NCH)]
        tT = [xs_pool.tile([128, NT], BF16, tag="tT", name=f"tT{c}") for c in range(NCH)]
        with ExitStack() as pr, nc.named_scope("p0_load"):
            pos_pool = pr.enter_context(tc.tile_pool(name="pos", bufs=4))
            spf_pool = pr.enter_context(tc.tile_pool(name="spf", bufs=1))
            sps_t = spf_pool.tile([128, NCH, APATCH], BF16, tag="sps")
            tps_t = spf_pool.tile([128, NCH, TS], BF16, tag="tps")
            spt_t = spf_pool.tile([128, NCH, VP], BF16, tag="spt")
            tpt_t = spf_pool.tile([128, NCH, T], BF16, tag="tpt")
            nc.scalar.dma_start(
                sps_t[:], sp_s[:].rearrange("(c p) n -> p c n", p=128))
            nc.scalar.dma_start(
                tps_t[:], tp_s[:].rearrange("(c p) n -> p c n", p=128))
            nc.scalar.dma_start(
                spt_t[:], sp_t[:].rearrange("(c p) n -> p c n", p=128))
            nc.scalar.dma_start(
                tpt_t[:], tp_t[:].rearrange("(c p) n -> p c n", p=128))

            def build_pos(pt, c, space_t, temp_t, nsp, ntp):
                a = space_t[:, c, :, None]
                b = temp_t[:, c, None, :]
                a2, b2 = bass.broadcast_tensor_aps(a, b)
                nc.vector.tensor_tensor(
                    pt[:].rearrange("p (n t) -> p n t", t=ntp), a2, b2, ADD)

            for c in range(NCH):
                sl = slice(c * 128, (c + 1) * 128)
                pt = pos_pool.tile([128, NT], BF16, tag="pos", name=f"pt{c}")
                nc.gpsimd.dma_start(tT[c][:], t_xT[sl, :])
                build_pos(pt, c, spt_t, tpt_t, VP, T)
                nc.vector.tensor_add(tT[c][:], tT[c][:], pt[:])
                nc.sync.dma_start(sT[c][:], s_xT[sl, :])
            for c in range(NCH):
                sl = slice(c * 128, (c + 1) * 128)
                nc.gpsimd.dma_start(kw[c][:], k_wT[sl, :])
                nc.sync.dma_start(qw[c][:], q_wT[sl, :])
            # K-proj chunk 0 fully — only needs tT; evacs land on the DVE
            # queue ahead of the s-side builds
            for (a, b) in QSL:
                emit_proj(kw, tT, kT, kb_t, 0, a, b)
            for c in range(NCH):
                ps2 = pos_pool.tile([128, NT], BF16, tag="pos", name=f"ps{c}")
                build_pos(ps2, c, sps_t, tps_t, APATCH, TS)
                nc.vector.tensor_add(sT[c][:], sT[c][:], ps2[:])
            for c in range(NCH):
                sl = slice(c * 128, (c + 1) * 128)
                nc.gpsimd.dma_start(vw[c][:], v_wT[sl, :])
                nc.scalar.dma_start(pw[c][:], proj_wT[sl, :])

        ms_eng = nc.gpsimd if V_MEMSET == "pool" else nc.vector
        for k in range(KT):
            ms_eng.memset(vP[k][:kw_of(k), :, :, 0:64], 1.0)

        P_pool = top.enter_context(tc.tile_pool(name="P", bufs=3))
        rcp_pool = top.enter_context(tc.tile_pool(name="rcp", bufs=2))
        ost_pool = top.enter_context(tc.tile_pool(name="ost", bufs=2))

        def emit_vproj(k):
            """V' for k-tile k. v_wT cols pre-reordered on host:
            group0 = even heads' v dims, group1 = odd heads'."""
            kwid = kw_of(k)
            for g in range(2):
                ps = gm_psum.tile([128, 512], F32, tag="gm")
                for c in range(NCH):
                    nc.tensor.matmul(
                        ps[:kwid, 0:384],
                        tT[c][:, k * 128:k * 128 + kwid],
                        vw[c][:, g * 384:(g + 1) * 384],
                        start=(c == 0), stop=(c == NCH - 1),
                    )
                dst = vP[k][:kwid, g, :, 64:128]
                src = ps[:kwid, 0:384].rearrange("p (h d) -> p h d", d=DH)
                bia = vb_t[:kwid, g * 384:(g + 1) * 384].rearrange(
                    "p (h d) -> p h d", d=DH)
                nc.vector.tensor_tensor(dst, src, bia, ADD)

        def emit_outproj(c_out, q0, qn):
            ps = gm_psum.tile([128, 512], F32, tag="gm")
            for c in range(NCH):
                nc.tensor.matmul(
                    ps[:, 0:qn],
                    pw[c][:, c_out * 128:(c_out + 1) * 128],
                    OT[c][:, q0:q0 + qn],
                    start=(c == 0), stop=(c == NCH - 1),
                )
            oe = ost_pool.tile([128, 512], BF16, tag="ost")
            nc.vector.tensor_scalar_add(
                oe[:, 0:qn], ps[:, 0:qn], pb_t[:, c_out:c_out + 1]
            )
            nc.sync.dma_start(outT[c_out * 128:(c_out + 1) * 128, q0:q0 + qn],
                              oe[:, 0:qn])

        filler = []  # deferred out-proj emissions (no forward PE deps)

        def filler_emit(n):
            for _ in range(n):
                if filler:
                    filler.pop(0)()

        def av_evac(c6, avA, avB, q0, qn):
            """Normalize + evac both heads of chunk c6 for q slice [q0, q0+qn)."""
            # both heads: den replicated at psum parts 0:64 (base-0 for the
            # custom DVE recip), O~ at 64:128; rcp written at base-0 SBUF.
            rcp = rcp_pool.tile([128, 1024], F32, tag="rcp")
            recip = (nc.vector.reciprocal_approx_fast if V_RECIP == "fast"
                     else nc.vector.reciprocal)
            recip(rcp[0:64, 0:qn], avA[0:64, 0:qn])
            recip(rcp[0:64, 512:512 + qn], avB[0:64, 0:qn])
            nc.vector.tensor_tensor(
                OT[c6][0:64, q0:q0 + qn], avA[64:128, 0:qn], rcp[0:64, 0:qn],
                MULT)
            nc.vector.tensor_tensor(
                OT[c6][64:128, q0:q0 + qn], avB[64:128, 0:qn],
                rcp[0:64, 512:512 + qn], MULT)

        def qk_mm(c6, ki, q0, qn):
            kwid = kw_of(ki)
            ksl = slice(ki * 128, ki * 128 + kwid)
            qk = qk_psum.tile([128, 1024], F32, tag="qk", name=f"qk{ki % 2}")
            nc.tensor.matmul(
                qk[:kwid, 0:qn], kT[c6][0:64, ksl], qT[c6][0:64, q0:q0 + qn],
                start=True, stop=True, tile_position=(0, 0),
            )
            nc.tensor.matmul(
                qk[:kwid, 512:512 + qn], kT[c6][64:128, ksl],
                qT[c6][64:128, q0:q0 + qn],
                start=True, stop=True, tile_position=(64, 0),
            )
            return qk

        # ---------------- main attention pipeline
        pend = []
        with nc.named_scope("attn"):
            if V_SERIAL:
                for k in range(KT):
                    emit_vproj(k)
            for qbi in range(NQB):
                q0 = qbi * QB
                for c6 in range(NCH):
                    # required projections for THIS (qbi, c6), in PE order
                    if V_SERIAL:
                        filler_emit(1)
                    if qbi == 0:
                        if V_SERIAL or V_QALL:
                            for (a, b) in QSL:
                                emit_proj(kw, tT, kT, kb_t, c6, a, b)
                            for (a, b) in (QSL if V_QALL else [(0, QB)]):
                                emit_proj(qw, sT, qT, qb_t, c6, a, b)
                        elif c6 == 0:
                            emit_proj(qw, sT, qT, qb_t, 0, 0, QB)
                            emit_vproj(0)
                            emit_vproj(1)
                        # seed next chunk's projections, paced inside ki loop
                        if not (V_SERIAL or V_QALL) and c6 + 1 < NCH:
                            for (a, b) in QSL:
                                pend.append(
                                    (lambda c=c6 + 1, a=a, b=b:
                                     emit_proj(kw, tT, kT, kb_t, c, a, b)))
                            pend.append(
                                (lambda c=c6 + 1:
                                 emit_proj(qw, sT, qT, qb_t, c, 0, QB)))
                    else:
                        if not V_QALL:
                            if qbi == 1:
                                emit_proj(qw, sT, qT, qb_t, c6, QB, QB)
                            if qbi == 2:
                                emit_proj(qw, sT, qT, qb_t, c6, 2 * QB, QB)
                        filler.append(
                            (lambda c=c6, a=(qbi - 1) * QB:
                             emit_outproj(c, a, QB)))

                    avA = av_psum.tile([128, 512], F32, tag="av", name="avA")
                    avB = av_psum.tile([128, 512], F32, tag="av", name="avB")

                    qk_next = qk_mm(c6, 0, q0, QB)
                    for ki in range(KT):
                        kwid = kw_of(ki)
                        qk = qk_next
                        if V_SERIAL:
                            pass
                        elif qbi == 0:
                            if c6 == 0:
                                if ki + 2 < KT:
                                    emit_vproj(ki + 2)
                                if pend and ki in (3, 5, 7, 9, 11, 12):
                                    pend.pop(0)()
                            elif pend and ki % 2 == 1:
                                pend.pop(0)()
                        elif filler and (ki % 4 == 3):
                            filler_emit(1)
                        if ki + 1 < KT:
                            qk_next = qk_mm(c6, ki + 1, q0, QB)
                        ex = P_pool.tile([128, 2, 512], BF16, tag="P")
                        nc.scalar.activation(
                            ex[:kwid, :, :],
                            qk[:kwid, :].rearrange("p (h q) -> p h q", h=2),
                            mybir.ActivationFunctionType.Exp,
                        )
                        if DBG and qbi == 0 and c6 == 0 and ki == 0:
                            nc.sync.dma_start(dbg_P[:], ex[:, :, :])
                        for hh in range(2):
                            nc.tensor.matmul(
                                (avA if hh == 0 else avB)[:, 0:QB],
                                vP[ki][:kwid, hh, c6, :],
                                ex[:kwid, hh, :],
                                start=(ki == 0), stop=(ki == KT - 1),
                            )
                    av_evac(c6, avA, avB, q0, QB)

            # ---- rump q block (32 cols): batch QK psum across all ki
            q0 = NQB * QB
            with nc.named_scope("rump"):
                for c6 in (range(NCH) if V_RUMP else []):
                    if not V_QALL:
                        emit_proj(qw, sT, qT, qb_t, c6, q0, QR)
                    avA = av_psum.tile([128, 512], F32, tag="av", name="avA")
                    avB = av_psum.tile([128, 512], F32, tag="av", name="avB")
                    qk = qk_psum.tile([128, 1024], F32, tag="qk", name="qkr")
                    # head-major, ki padded to 16: head A fills psum bank 0,
                    # head B bank 1 (concurrent pair must hit distinct banks)
                    qkv = qk[:, :].rearrange("p (h k q) -> p h k q", h=2, k=16)
                    for ki in range(KT):
                        kwid = kw_of(ki)
                        ksl = slice(ki * 128, ki * 128 + kwid)
                        nc.tensor.matmul(
                            qkv[:kwid, 0, ki, :], kT[c6][0:64, ksl],
                            qT[c6][0:64, q0:q0 + QR],
                            start=True, stop=True, tile_position=(0, 0),
                        )
                        nc.tensor.matmul(
                            qkv[:kwid, 1, ki, :], kT[c6][64:128, ksl],
                            qT[c6][64:128, q0:q0 + QR],
                            start=True, stop=True, tile_position=(64, 0),
                        )
                    ex = P_pool.tile([128, 2, KT, QR], BF16, tag="Pr")
                    for hh in range(2):
                        nc.scalar.activation(
                            ex[:, hh, 0:12, :], qkv[:, hh, 0:12, :],
                            mybir.ActivationFunctionType.Exp,
                        )
                        nc.scalar.activation(
                            ex[0:KR, hh, 12, :], qkv[0:KR, hh, 12, :],
                            mybir.ActivationFunctionType.Exp,
                        )
                    filler_emit(2)
                    for ki in range(KT):
                        kwid = kw_of(ki)
                        for hh in range(2):
                            nc.tensor.matmul(
                                (avA if hh == 0 else avB)[:, 0:QR],
                                vP[ki][:kwid, hh, c6, :],
                                ex[:kwid, hh, ki, :],
                                start=(ki == 0), stop=(ki == KT - 1),
                            )
                    av_evac(c6, avA, avB, q0, QR)
                    filler.append(
                        (lambda c=c6, a=(NQB - 1) * QB:
                         emit_outproj(c, a, QB)))

            if DBG:
                nc.sync.dma_start(dbg_sT[:], sT[0][:])
                for c in range(NCH):
                    nc.sync.dma_start(dbg_qT[c], qT[c][:])
                nc.sync.dma_start(dbg_kT[:], kT[0][:])
                nc.sync.dma_start(dbg_vP[:], vP[0][:])
                for c in range(NCH):
                    nc.sync.dma_start(dbg_OT[c], OT[c][:])
            # ---- drain remaining filler + final out-proj slices
            with nc.named_scope("tail"):
                filler_emit(len(filler))
                if V_RUMP:
                    for c6 in range(NCH):
                        emit_outproj(c6, NQB * QB, QR)

    nc.finalize()
    return nc


def _install_axon_ntff_shim():
    if "antenv.axon_hooks" in sys.modules:
        return
    mod = types.ModuleType("antenv.axon_hooks")
    mod._hook = None
    mod.set_axon_ntff_profile_hook = lambda h: setattr(mod, "_hook", h)
    mod.get_axon_ntff_profile_hook = lambda: mod._hook
    sys.modules["antenv.axon_hooks"] = mod
    try:
        import antenv

        antenv.axon_hooks = mod
        from trn_agent_boot.trn_boot import _ntff_profile_via_ctypes

        hook = _ntff_profile_via_ctypes("/opt/axon/libaxon_pjrt.so")
        if hook is not None:
            mod.set_axon_ntff_profile_hook(hook)
    except Exception:
        pass


def prep_inputs(s_x, t_x, clip_space_pos, vmae_space_pos, clip_temporal_pos,
                vmae_temporal_pos, q_w, q_b, kv_w, kv_b, proj_w, proj_b):
    """Host-side sharding/layout prep. Returns list of 8 per-core input maps."""
    f = np.float32
    bf = ml_dtypes.bfloat16
    sp_s = np.ascontiguousarray(np.asarray(clip_space_pos).T).astype(bf)
    tp_s = np.ascontiguousarray(np.asarray(clip_temporal_pos).T).astype(bf)
    sp_t = np.ascontiguousarray(np.asarray(vmae_space_pos).T).astype(bf)
    tp_t = np.ascontiguousarray(np.asarray(vmae_temporal_pos).T).astype(bf)
    q_wT = np.ascontiguousarray(np.asarray(q_w).T * SCALE).astype(bf)
    k_wT = np.ascontiguousarray(np.asarray(kv_w)[:DIM].T).astype(bf)
    # v weight rows reordered: [even heads' v dims | odd heads' v dims]
    v_w = np.asarray(kv_w)[DIM:]         # [768 out, 768 in]
    v_b = np.asarray(kv_b)[DIM:]
    order = np.concatenate([
        np.arange(DIM).reshape(H, DH)[0::2].reshape(-1),
        np.arange(DIM).reshape(H, DH)[1::2].reshape(-1),
    ])
    v_wT = np.ascontiguousarray(v_w[order].T).astype(bf)
    v_br = np.ascontiguousarray(
        np.broadcast_to(v_b[order].reshape(1, DIM), (128, DIM)), dtype=f)
    proj_wT = np.ascontiguousarray(np.asarray(proj_w).T).astype(bf)
    q_b2 = np.ascontiguousarray(
        (np.asarray(q_b) * SCALE).reshape(NCH, 128).T, dtype=f)
    k_b2 = np.ascontiguousarray(
        np.asarray(kv_b)[:DIM].reshape(NCH, 128).T, dtype=f)
    p_b2 = np.ascontiguousarray(np.asarray(proj_b).reshape(NCH, 128).T, dtype=f)

    in_maps = []
    for b in range(B):
        s_slice = np.asarray(s_x)[:, b * TS:(b + 1) * TS, :]  # (196, 8, 768)
        t_slice = np.asarray(t_x)[1:, b * T:(b + 1) * T, :]   # (196, 8, 768)
        s_xT = np.ascontiguousarray(
            s_slice.transpose(2, 0, 1).reshape(DIM, NT)).astype(bf)
        t_xT = np.ascontiguousarray(
            t_slice.transpose(2, 0, 1).reshape(DIM, NT)).astype(bf)
        in_maps.append({
            "s_xT": s_xT, "t_xT": t_xT,
            "sp_s": sp_s, "tp_s": tp_s, "sp_t": sp_t, "tp_t": tp_t,
            "q_wT": q_wT, "k_wT": k_wT, "v_wT": v_wT, "proj_wT": proj_wT,
            "q_b2": q_b2, "k_b2": k_b2, "p_b2": p_b2, "v_br": v_br,
        })
    return in_maps


def unshard_output(results):
    """results: list of 8 dicts with 'outT' [768, 1568] -> (196, 64, 768)."""
    out = np.empty((APATCH, B * TS, DIM), dtype=np.float32)
    for b in range(B):
        o = results[b]["outT"].astype(np.float32).reshape(DIM, APATCH, TS)
        out[:, b * TS:(b + 1) * TS, :] = o.transpose(1, 2, 0)
    return out


def kernel(**inputs):
    _install_axon_ntff_shim()
    in_maps = prep_inputs(**inputs)
    if "nc" not in _NC_CACHE:
        _NC_CACHE["nc"] = build_nc()
    nc = _NC_CACHE["nc"]
    res = run_bass_kernel_spmd(nc, in_maps, core_ids=list(range(B)))
    return unshard_output(res.results)


if __name__ == "__main__":
    rng = np.random.default_rng(0)
    fake = {
        "s_x": rng.standard_normal((APATCH, B * TS, DIM), dtype=np.float32),
        "t_x": rng.standard_normal((VP + 1, B * T, DIM), dtype=np.float32),
        "clip_space_pos": SCALE * rng.standard_normal((APATCH, DIM), dtype=np.float32),
        "vmae_space_pos": SCALE * rng.standard_normal((VP, DIM), dtype=np.float32),
        "clip_temporal_pos": SCALE * rng.standard_normal((TS, DIM), dtype=np.float32),
        "vmae_temporal_pos": SCALE * rng.standard_normal((T, DIM), dtype=np.float32),
        "q_w": (0.02 * rng.standard_normal((DIM, DIM))).astype(np.float32),
        "q_b": np.zeros(DIM, np.float32),
        "kv_w": (0.02 * rng.standard_normal((2 * DIM, DIM))).astype(np.float32),
        "kv_b": np.zeros(2 * DIM, np.float32),
        "proj_w": (0.02 * rng.standard_normal((DIM, DIM))).astype(np.float32),
        "proj_b": np.zeros(DIM, np.float32),
    }
    out = kernel(**fake)
    print("out", out.shape, out.dtype)
